# revision 22
# baseline (speedup 1.0000x reference)
import os
import sys

sys.path.insert(0, '/opt/trn_rl_repo')
import numpy as np

NCORES = 8
N = 100000
H = 128
H2 = 2 * H                     # gather element: 2 nodes per pair-row
GROUPS = 4
GC = H // GROUPS
K = 5
SHARD_N = N // NCORES          # 12500 owned real nodes per core
TILES = 98
SHARD = TILES * 128            # 12544 padded
FULL = SHARD * NCORES          # 100352
NWIN = 2                       # pair-row windows: cores 0-3 / cores 4-7
PWIN = FULL // 2 // NWIN       # 25088 pair-rows per window (int16-safe)
NTMAX = 12                     # max tiles per chunk
CAPR = 56                      # max gather rounds (pair fetches) per chunk
RMS_EPS = 1.1920929e-07

_cache = {}


def _build(schedule, repeat=1):
    from concourse import bacc, mybir, tile

    (chunks, Q, col0w, TR) = schedule
    # chunks: list of [consecutive tile indices]; Q: [nchunk, NWIN] rounds
    # per tile per window; col0w: [nchunk, NWIN] global start column of
    # each (chunk, window) gather call; TR: total columns.
    f32 = mybir.dt.float32
    bf16 = mybir.dt.bfloat16
    i16 = mybir.dt.int16

    key = ("v4", TR, os.environ.get("KERNEL_ABLATE"),
           tuple(Q.ravel()), tuple(tuple(ts) for ts in chunks), repeat)
    if key in _cache:
        return _cache[key]

    ablate = os.environ.get("KERNEL_ABLATE")
    nc = bacc.Bacc("TRN2", target_bir_lowering=False, debug=False,
                   num_devices=NCORES)
    xfb = nc.dram_tensor("x_fullb", [FULL, H], bf16, kind="ExternalInput")
    xo = nc.dram_tensor("x_own", [SHARD, H], f32, kind="ExternalInput")
    idx_in = nc.dram_tensor("idx_in", [128, 8 * TR], i16, kind="ExternalInput")
    ew_in = nc.dram_tensor("ew_in", [128, 2 * TR], bf16, kind="ExternalInput")
    par_in = nc.dram_tensor("par_in", [128, 40], f32, kind="ExternalInput")
    rw_in = nc.dram_tensor("rw_in", [128, H], f32, kind="ExternalInput")
    out_ext = nc.dram_tensor("out", [SHARD, H], f32, kind="ExternalOutput")

    nch = len(chunks)

    with tile.TileContext(nc) as tc:
        with (
            tc.tile_pool(name="dram", bufs=1, space="DRAM") as dram,
            tc.tile_pool(name="big", bufs=1) as big,
            tc.tile_pool(name="stage", bufs=2) as stage,
            tc.tile_pool(name="small", bufs=1) as small,
        ):
            idx_sb = big.tile([128, 8 * TR], i16, tag="idx")
            ew_sb = big.tile([128, 2 * TR], bf16, tag="ew")
            par_sb = big.tile([128, 40], f32, tag="par")
            rw_sb = big.tile([128, H], f32, tag="rw")
            nc.sync.dma_start(idx_sb[:], idx_in[:])
            nc.sync.dma_start(ew_sb[:], ew_in[:])
            nc.sync.dma_start(par_sb[:], par_in[:])
            nc.sync.dma_start(rw_sb[:], rw_in[:])

            TA = big.tile([128, SHARD], f32, tag="TA")
            TB = big.tile([128, SHARD], f32, tag="TB")
            res = big.tile([128, SHARD], bf16, tag="res")
            propb = big.tile([128, NTMAX * H], f32, tag="prop")
            tmpb = big.tile([128, NTMAX * H], f32, tag="tmp")

            def sl(ap, t0, t1):  # [128, SHARD] -> [128, (t1-t0)*H] slice
                return ap[:, t0 * H:t1 * H]

            def cheb(k):  # [128, 4] broadcast-ready column block for hop k
                return par_sb[:, 4 * k:4 * k + 4]

            def gv(ap2, nt):  # [128, nt*H] -> [128, nt, 4, GC] group view
                return ap2.rearrange("p (t g c) -> p t g c", g=GROUPS, c=GC)

            def cbc(k, nt):  # cheb col k broadcast to [128, nt, 4, GC]
                return cheb(k).rearrange("p g -> p () g ()").to_broadcast(
                    [128, nt, GROUPS, GC])

            for rep in range(repeat):
                sfx = f"_{rep}" if rep else ""
                shard_b = [dram.tile([SHARD, H], bf16, tag=f"sh{k}{sfx}",
                                     name=f"sh{k}{sfx}") for k in range(4)]
                full_b = [dram.tile([FULL, H], bf16, tag=f"fl{k}{sfx}",
                                    name=f"fl{k}{sfx}", addr_space="Shared")
                          for k in range(4)]
                # T_prev2 = x (fp32, SBUF resident)
                nc.sync.dma_start(
                    TB[:].rearrange("p (t j) -> p t j", j=H),
                    xo[:].rearrange("(t p) j -> p t j", p=128))

                for hop in range(1, K + 1):
                    tabt = xfb if hop == 1 else full_b[hop - 2]
                    # pair-row view: [FULL/2, 2H]; window w covers pair
                    # rows [w*PWIN, (w+1)*PWIN) = cores 4w..4w+3
                    src_tab = tabt[:].rearrange("(r f) h -> r (f h)", f=2)
                    # hop k >= 2 overwrites the buffer holding T_{k-2}
                    cur = TA if hop % 2 == 1 else TB
                    for s in range(nch):
                        ts = chunks[s]
                        nt = len(ts)
                        t0, t1 = ts[0], ts[-1] + 1
                        rtot = int(nt * Q[s].sum())
                        gbuf = stage.tile([128, CAPR, H2], bf16, tag="g")
                        base = 0
                        for w in range(NWIN):
                            qw = int(Q[s, w])
                            if qw == 0:
                                continue
                            rw_ = nt * qw
                            nidx = rw_ * 128
                            cb = int(col0w[s, w])
                            if ablate != "gather":
                                nc.gpsimd.dma_gather(
                                    gbuf[:, base:base + rw_, :],
                                    src_tab[w * PWIN:(w + 1) * PWIN, :],
                                    idx_sb[:, 8 * cb:8 * (cb + rw_)],
                                    nidx, nidx, H2, single_packet=False,
                                )
                            base += rw_
                        cb0 = int(col0w[s, 0])
                        # bulk ew multiply on Pool so it overlaps the
                        # DVE reduces of the previous chunk
                        nc.gpsimd.tensor_mul(
                            gbuf[:, :rtot, :]
                            .rearrange("p r (f h) -> p (r f) h", f=2),
                            gbuf[:, :rtot, :]
                            .rearrange("p r (f h) -> p (r f) h", f=2),
                            ew_sb[:, 2 * cb0:2 * (cb0 + rtot)]
                            .rearrange("p r -> p r ()")
                            .to_broadcast([128, 2 * rtot, H]))
                        # per-window segmented reduce over (round, sub)
                        ptgts = [(sl(cur[:], t0, t1) if hop == 1
                                  else propb[:, :nt * H]),
                                 tmpb[:, :nt * H]]
                        base = 0
                        for w in range(NWIN):
                            qw = int(Q[s, w])
                            if qw == 0:
                                nc.vector.memset(ptgts[w], 0.0)
                                continue
                            rw_ = nt * qw
                            if ablate == "dve":
                                nc.vector.memset(ptgts[w], 0.0)
                                base += rw_
                                continue
                            nc.vector.tensor_reduce(
                                ptgts[w].rearrange("p (t h) -> p t h", h=H),
                                gbuf[:, base:base + rw_, :]
                                .rearrange("p (t r) (f h) -> p t h (r f)",
                                           r=qw, f=2),
                                mybir.AxisListType.X, mybir.AluOpType.add)
                            base += rw_
                        # combine windows: ptgts[0] += ptgts[1]
                        nc.vector.tensor_add(ptgts[0], ptgts[0], ptgts[1])
                        if hop >= 2:
                            # T_k = 2*prop - T_{k-2} (in place over T_{k-2})
                            nc.vector.scalar_tensor_tensor(
                                sl(cur[:], t0, t1), propb[:, :nt * H], 2.0,
                                sl(cur[:], t0, t1),
                                mybir.AluOpType.mult,
                                mybir.AluOpType.subtract)
                            # res += c_hop * T_k
                            nc.vector.tensor_mul(
                                gv(tmpb[:, :nt * H], nt),
                                gv(sl(cur[:], t0, t1), nt), cbc(hop, nt))
                            nc.vector.tensor_add(
                                sl(res[:], t0, t1), sl(res[:], t0, t1),
                                tmpb[:, :nt * H])
                        else:
                            # res = c0*x + c1*T1  (TB holds x)
                            nc.vector.tensor_mul(
                                gv(tmpb[:, :nt * H], nt),
                                gv(sl(cur[:], t0, t1), nt), cbc(1, nt))
                            nc.vector.tensor_mul(
                                gv(sl(res[:], t0, t1), nt),
                                gv(sl(TB[:], t0, t1), nt), cbc(0, nt))
                            nc.vector.tensor_add(
                                sl(res[:], t0, t1), sl(res[:], t0, t1),
                                tmpb[:, :nt * H])
                        # store T_k chunk as bf16 (cast DMA) for AllGather
                        if hop <= 4:
                            nc.gpsimd.dma_start(
                                shard_b[hop - 1][t0 * 128:t1 * 128, :]
                                .rearrange("(t p) j -> p t j", p=128),
                                sl(cur[:], t0, t1)
                                .rearrange("p (t j) -> p t j", j=H))
                    if hop <= 4:
                        nc.gpsimd.collective_compute(
                            "AllGather", mybir.AluOpType.bypass,
                            replica_groups=[list(range(NCORES))],
                            ins=[shard_b[hop - 1][:].opt()],
                            outs=[full_b[hop - 1][:].opt()],
                        )

                # epilogue: res complete; TA/TB are dead scratch now
                def res3(ap):
                    return ap.rearrange("p (t j) -> p t j", j=H)

                gall = res[:].rearrange("p (t g c) -> p t g c", g=GROUPS,
                                        c=GC)

                def pbc(c0):
                    return (par_sb[:, c0:c0 + 4]
                            .rearrange("p g -> p () g ()")
                            .to_broadcast([128, TILES, GROUPS, GC]))

                nc.vector.tensor_mul(gall, gall, pbc(24))
                nc.vector.tensor_add(gall, gall, pbc(28))
                nc.vector.tensor_mul(TB[:], res[:], res[:])
                ssq = small.tile([128, TILES], f32, tag="ssq")
                nc.vector.tensor_reduce(
                    ssq[:], res3(TB[:]), mybir.AxisListType.X,
                    mybir.AluOpType.add)
                rms = small.tile([128, TILES], f32, tag="rms")
                sq = small.tile([128, TILES], f32, tag="sqr")
                nc.scalar.activation(
                    sq[:], ssq[:], mybir.ActivationFunctionType.Sqrt,
                    bias=par_sb[:, 32:33], scale=1.0 / H)
                nc.vector.reciprocal(rms[:], sq[:])
                nc.vector.tensor_mul(
                    res3(TA[:]), res3(res[:]),
                    rms[:].rearrange("p (t o) -> p t o", o=1)
                    .to_broadcast([128, TILES, H]))
                nc.vector.tensor_mul(
                    res3(TA[:]), res3(TA[:]),
                    rw_sb[:].rearrange("p (o j) -> p o j", o=1)
                    .to_broadcast([128, TILES, H]))
                # SiLU = x * sigmoid(x)
                nc.scalar.activation(
                    TB[:], TA[:], mybir.ActivationFunctionType.Sigmoid)
                nc.vector.tensor_mul(TA[:], TA[:], TB[:])
                nc.sync.dma_start(
                    out_ext[:].rearrange("(t p) j -> p t j", p=128),
                    res3(TA[:]))

    nc.compile()
    _cache[key] = nc
    return nc


def _order_profiles(profs, counts):
    """Greedy nearest-neighbor chain over distinct profile rows (L1)."""
    np_, _ = profs.shape
    visited = np.zeros(np_, bool)
    cur = int(np.argmax(counts))
    order = [cur]
    visited[cur] = True
    for _ in range(np_ - 1):
        d = np.abs(profs - profs[cur]).sum(axis=1).astype(np.float64)
        d[visited] = np.inf
        cur = int(np.argmin(d))
        order.append(cur)
        visited[cur] = True
    return np.array(order)


def _prep(x, edge_weight_norm, edge_index):
    src = np.asarray(edge_index[0]).astype(np.int64)
    dst = np.asarray(edge_index[1]).astype(np.int64)
    ew = np.asarray(edge_weight_norm, dtype=np.float32)
    E = src.shape[0]

    # pass 1: degree-sorted round-robin deal fixes each node's core (and
    # hence its pair-window = core//4 as a gather source)
    deg = np.bincount(dst, minlength=N)
    order1 = np.argsort(-deg, kind='stable')
    node_core = np.empty(N, np.int64)
    node_core[order1] = np.arange(N) % NCORES

    # profiles: in-edge counts split by src window (fixed by pass 1)
    src_win = node_core[src] // 4
    prof = np.zeros((N, NWIN), np.int32)
    np.add.at(prof, (dst, src_win), 1)

    # pass 2: within each core, order nodes by a shared profile-bucket
    # chain so tiles hold near-identical (c0, c1) profiles, aligned
    # across cores by quantile
    uniq, uinv, ucnt = np.unique(prof, axis=0, return_inverse=True,
                                 return_counts=True)
    porder = _order_profiles(uniq.astype(np.int64), ucnt)
    prank = np.empty(len(uniq), np.int64)
    prank[porder] = np.arange(len(uniq))
    node_rank = prank[uinv]

    perm_pos = np.empty(N, np.int64)
    for c in range(NCORES):
        nodes = np.flatnonzero(node_core == c)
        o = np.argsort(node_rank[nodes], kind='stable')
        perm_pos[nodes[o]] = c * SHARD + np.arange(len(nodes))

    dst_p = perm_pos[dst]
    dst_core = dst_p // SHARD
    dst_local = dst_p - dst_core * SHARD
    tile_id = dst_local // 128
    part_id = dst_local % 128
    src_p = perm_pos[src]
    prow = src_p // 2
    win_id = prow // PWIN
    win_rel = (prow - win_id * PWIN).astype(np.int16)
    sub = src_p % 2

    cnt = np.zeros((NCORES, TILES, 128, NWIN), np.int32)
    np.add.at(cnt, (dst_core, tile_id, part_id, win_id), 1)
    R_tw = cnt.max(axis=(0, 2)).astype(np.int64)  # [TILES, NWIN]

    # greedy chunks of consecutive tiles, uniform per-window rounds
    chunks = []
    q_list = []
    cur = [0]
    q = R_tw[0].copy()
    for t in range(1, TILES):
        q2 = np.maximum(q, R_tw[t])
        nt = len(cur)
        waste = (nt + 1) * q2.sum() - (nt * q.sum() + R_tw[t].sum())
        if nt + 1 <= NTMAX and (nt + 1) * q2.sum() <= CAPR and waste <= 3:
            cur.append(t)
            q = q2
        else:
            chunks.append(cur)
            q_list.append(q)
            cur = [t]
            q = R_tw[t].copy()
    chunks.append(cur)
    q_list.append(q)
    Q = np.stack(q_list)  # [nchunk, NWIN]
    nch = len(chunks)
    col0w = np.zeros((nch, NWIN), np.int64)
    run = 0
    for s in range(nch):
        for w in range(NWIN):
            col0w[s, w] = run
            run += len(chunks[s]) * int(Q[s, w])
    TR = int(run)

    chunk_of = np.zeros(TILES, np.int64)
    tl_of = np.zeros(TILES, np.int64)
    for s, ts in enumerate(chunks):
        for i, t in enumerate(ts):
            chunk_of[t] = s
            tl_of[t] = i

    # rank of edge within its (core, tile, part, window) group
    key = (((dst_core * TILES + tile_id) * 128 + part_id) * NWIN + win_id)
    o = np.argsort(key, kind='stable')
    ks = key[o]
    starts = np.r_[0, np.flatnonzero(np.diff(ks)) + 1]
    group_len = np.diff(np.r_[starts, E])
    rank_sorted = np.arange(E) - np.repeat(starts, group_len)
    erank = np.empty(E, np.int64)
    erank[o] = rank_sorted

    s_of = chunk_of[tile_id]
    gcol = (col0w[s_of, win_id] + tl_of[tile_id] * Q[s_of, win_id] + erank)

    ew_all = []
    idxw_all = []
    for c in range(NCORES):
        m = dst_core == c
        ewf = np.zeros((128, 2 * TR), np.float32)
        ewf[part_id[m], 2 * gcol[m] + sub[m]] = ew[m]
        idx_flat = np.zeros(TR * 128, np.int16)
        idx_flat[gcol[m] * 128 + part_id[m]] = win_rel[m]
        iw = np.zeros((128, 8 * TR), np.int16)
        for s in range(nch):
            nt = len(chunks[s])
            for w in range(NWIN):
                rw_ = nt * int(Q[s, w])
                if rw_ == 0:
                    continue
                cb = int(col0w[s, w])
                seg = idx_flat[cb * 128:(cb + rw_) * 128]
                iw[:, 8 * cb:8 * (cb + rw_)] = np.tile(
                    seg.reshape(-1, 16).T, (8, 1))
        ew_all.append(ewf)
        idxw_all.append(iw)

    x_full = np.zeros((FULL, H), np.float32)
    x_full[perm_pos] = np.asarray(x, np.float32)
    x_own_all = [x_full[c * SHARD:(c + 1) * SHARD] for c in range(NCORES)]
    inv = np.full(FULL, -1, np.int64)
    inv[perm_pos] = np.arange(N)
    schedule = (chunks, Q, col0w, TR)
    return x_full, x_own_all, idxw_all, ew_all, schedule, inv


def _np_bf16(a):
    import ml_dtypes
    return np.asarray(a, np.float32).astype(ml_dtypes.bfloat16)


def kernel(x, edge_weight_norm, cheb_coeffs, group_scale, group_bias,
           rms_weight, edge_index):
    from concourse.bass_utils import run_bass_kernel_spmd

    x = np.asarray(x, np.float32)
    assert x.shape == (N, H)
    x_full, x_own_all, idxw_all, ew_all, schedule, inv = _prep(
        x, edge_weight_norm, edge_index)

    params = np.zeros((128, 40), np.float32)
    params[:, 32] = RMS_EPS
    cheb = np.asarray(cheb_coeffs, np.float32)      # [4, K+1]
    params[:, :24] = cheb.T.reshape(1, 24)          # k-major: col = 4k+g
    params[:, 24:28] = np.asarray(group_scale, np.float32).reshape(1, 4)
    params[:, 28:32] = np.asarray(group_bias, np.float32).reshape(1, 4)
    rmsw = np.tile(np.asarray(rms_weight, np.float32).reshape(1, H), (128, 1))

    repeat = int(os.environ.get("KERNEL_REPEAT", "1"))
    nc = _build(schedule, repeat=repeat)

    x_fullb = _np_bf16(x_full)
    in_maps = []
    for c in range(NCORES):
        in_maps.append({
            "x_fullb": x_fullb,
            "x_own": x_own_all[c],
            "idx_in": idxw_all[c],
            "ew_in": _np_bf16(ew_all[c]),
            "par_in": params,
            "rw_in": rmsw,
        })
    res = run_bass_kernel_spmd(nc, in_maps, list(range(NCORES)))
    out_shards = np.stack([res.results[c]["out"] for c in range(NCORES)],
                          axis=0)
    out_full = out_shards.reshape(FULL, H)
    out = np.empty((N, H), np.float32)
    mask = inv >= 0
    out[inv[mask]] = out_full[mask]
    return out


# revision 23
# speedup vs baseline: 11.7308x; 11.7308x over previous
import os
import sys

sys.path.insert(0, '/opt/trn_rl_repo')
import numpy as np

NCORES = 8
N = 100000
H = 128
H2 = 2 * H                     # gather element: 2 nodes per pair-row
GROUPS = 4
GC = H // GROUPS
K = 5
SHARD_N = N // NCORES          # 12500 owned real nodes per core
TILES = 98
SHARD = TILES * 128            # 12544 padded
FULL = SHARD * NCORES          # 100352
NWIN = 2                       # pair-row windows: cores 0-3 / cores 4-7
PWIN = FULL // 2 // NWIN       # 25088 pair-rows per window (int16-safe)
NTMAX = 12                     # max tiles per chunk
CAPR = 56                      # max gather rounds (pair fetches) per chunk
RMS_EPS = 1.1920929e-07

_cache = {}


def _build(schedule, repeat=1):
    from concourse import bacc, mybir, tile

    (chunks, Q, col0w, TR) = schedule
    # chunks: list of [consecutive tile indices]; Q: [nchunk, NWIN] rounds
    # per tile per window; col0w: [nchunk, NWIN] global start column of
    # each (chunk, window) gather call; TR: total columns.
    f32 = mybir.dt.float32
    bf16 = mybir.dt.bfloat16
    i16 = mybir.dt.int16

    key = ("v4", TR, os.environ.get("KERNEL_ABLATE"),
           tuple(Q.ravel()), tuple(tuple(ts) for ts in chunks), repeat)
    if key in _cache:
        return _cache[key]

    ablate = os.environ.get("KERNEL_ABLATE")
    nc = bacc.Bacc("TRN2", target_bir_lowering=False, debug=False,
                   num_devices=NCORES)
    xfb = nc.dram_tensor("x_fullb", [FULL, H], bf16, kind="ExternalInput")
    xo = nc.dram_tensor("x_own", [SHARD, H], f32, kind="ExternalInput")
    idx_in = nc.dram_tensor("idx_in", [128, 8 * TR], i16, kind="ExternalInput")
    ew_in = nc.dram_tensor("ew_in", [128, 2 * TR], bf16, kind="ExternalInput")
    par_in = nc.dram_tensor("par_in", [128, 40], f32, kind="ExternalInput")
    rw_in = nc.dram_tensor("rw_in", [128, H], f32, kind="ExternalInput")
    out_ext = nc.dram_tensor("out", [SHARD, H], f32, kind="ExternalOutput")

    nch = len(chunks)

    with tile.TileContext(nc) as tc:
        with (
            tc.tile_pool(name="dram", bufs=1, space="DRAM") as dram,
            tc.tile_pool(name="big", bufs=1) as big,
            tc.tile_pool(name="stage", bufs=2) as stage,
            tc.tile_pool(name="small", bufs=1) as small,
        ):
            idx_sb = big.tile([128, 8 * TR], i16, tag="idx")
            ew_sb = big.tile([128, 2 * TR], bf16, tag="ew")
            par_sb = big.tile([128, 40], f32, tag="par")
            rw_sb = big.tile([128, H], f32, tag="rw")
            nc.sync.dma_start(idx_sb[:], idx_in[:])
            nc.sync.dma_start(ew_sb[:], ew_in[:])
            nc.sync.dma_start(par_sb[:], par_in[:])
            nc.sync.dma_start(rw_sb[:], rw_in[:])

            TA = big.tile([128, SHARD], f32, tag="TA")
            TB = big.tile([128, SHARD], f32, tag="TB")
            res = big.tile([128, SHARD], bf16, tag="res")
            propb = big.tile([128, NTMAX * H], f32, tag="prop")
            tmpb = big.tile([128, NTMAX * H], f32, tag="tmp")

            def sl(ap, t0, t1):  # [128, SHARD] -> [128, (t1-t0)*H] slice
                return ap[:, t0 * H:t1 * H]

            def cheb(k):  # [128, 4] broadcast-ready column block for hop k
                return par_sb[:, 4 * k:4 * k + 4]

            def gv(ap2, nt):  # [128, nt*H] -> [128, nt, 4, GC] group view
                return ap2.rearrange("p (t g c) -> p t g c", g=GROUPS, c=GC)

            def cbc(k, nt):  # cheb col k broadcast to [128, nt, 4, GC]
                return cheb(k).rearrange("p g -> p () g ()").to_broadcast(
                    [128, nt, GROUPS, GC])

            for rep in range(repeat):
                sfx = f"_{rep}" if rep else ""
                shard_b = [dram.tile([SHARD, H], bf16, tag=f"sh{k}{sfx}",
                                     name=f"sh{k}{sfx}") for k in range(4)]
                full_b = [dram.tile([FULL, H], bf16, tag=f"fl{k}{sfx}",
                                    name=f"fl{k}{sfx}", addr_space="Shared")
                          for k in range(4)]
                # T_prev2 = x (fp32, SBUF resident)
                nc.sync.dma_start(
                    TB[:].rearrange("p (t j) -> p t j", j=H),
                    xo[:].rearrange("(t p) j -> p t j", p=128))

                for hop in range(1, K + 1):
                    tabt = xfb if hop == 1 else full_b[hop - 2]
                    # pair-row view: [FULL/2, 2H]; window w covers pair
                    # rows [w*PWIN, (w+1)*PWIN) = cores 4w..4w+3
                    src_tab = tabt[:].rearrange("(r f) h -> r (f h)", f=2)
                    # hop k >= 2 overwrites the buffer holding T_{k-2}
                    cur = TA if hop % 2 == 1 else TB
                    for s in range(nch):
                        ts = chunks[s]
                        nt = len(ts)
                        t0, t1 = ts[0], ts[-1] + 1
                        rtot = int(nt * Q[s].sum())
                        gbuf = stage.tile([128, CAPR, H2], bf16, tag="g")
                        base = 0
                        for w in range(NWIN):
                            qw = int(Q[s, w])
                            if qw == 0:
                                continue
                            rw_ = nt * qw
                            nidx = rw_ * 128
                            cb = int(col0w[s, w])
                            if ablate != "gather":
                                nc.gpsimd.dma_gather(
                                    gbuf[:, base:base + rw_, :],
                                    src_tab[w * PWIN:(w + 1) * PWIN, :],
                                    idx_sb[:, 8 * cb:8 * (cb + rw_)],
                                    nidx, nidx, H2, single_packet=False,
                                )
                            base += rw_
                        cb0 = int(col0w[s, 0])
                        # bulk ew multiply: per sub-node scalar over H
                        nc.vector.tensor_mul(
                            gbuf[:, :rtot, :]
                            .rearrange("p r (f h) -> p (r f) h", f=2),
                            gbuf[:, :rtot, :]
                            .rearrange("p r (f h) -> p (r f) h", f=2),
                            ew_sb[:, 2 * cb0:2 * (cb0 + rtot)]
                            .rearrange("p r -> p r ()")
                            .to_broadcast([128, 2 * rtot, H]))
                        # per-window segmented reduce over (round, sub)
                        ptgts = [(sl(cur[:], t0, t1) if hop == 1
                                  else propb[:, :nt * H]),
                                 tmpb[:, :nt * H]]
                        base = 0
                        for w in range(NWIN):
                            qw = int(Q[s, w])
                            if qw == 0:
                                nc.vector.memset(ptgts[w], 0.0)
                                continue
                            rw_ = nt * qw
                            if ablate == "dve":
                                nc.vector.memset(ptgts[w], 0.0)
                                base += rw_
                                continue
                            nc.vector.tensor_reduce(
                                ptgts[w].rearrange("p (t h) -> p t h", h=H),
                                gbuf[:, base:base + rw_, :]
                                .rearrange("p (t r) (f h) -> p t h (r f)",
                                           r=qw, f=2),
                                mybir.AxisListType.X, mybir.AluOpType.add)
                            base += rw_
                        # combine windows: ptgts[0] += ptgts[1]
                        nc.vector.tensor_add(ptgts[0], ptgts[0], ptgts[1])
                        if hop >= 2:
                            # T_k = 2*prop - T_{k-2} (in place over T_{k-2})
                            nc.vector.scalar_tensor_tensor(
                                sl(cur[:], t0, t1), propb[:, :nt * H], 2.0,
                                sl(cur[:], t0, t1),
                                mybir.AluOpType.mult,
                                mybir.AluOpType.subtract)
                            # res += c_hop * T_k
                            nc.vector.tensor_mul(
                                gv(tmpb[:, :nt * H], nt),
                                gv(sl(cur[:], t0, t1), nt), cbc(hop, nt))
                            nc.vector.tensor_add(
                                sl(res[:], t0, t1), sl(res[:], t0, t1),
                                tmpb[:, :nt * H])
                        else:
                            # res = c0*x + c1*T1  (TB holds x)
                            nc.vector.tensor_mul(
                                gv(tmpb[:, :nt * H], nt),
                                gv(sl(cur[:], t0, t1), nt), cbc(1, nt))
                            nc.vector.tensor_mul(
                                gv(sl(res[:], t0, t1), nt),
                                gv(sl(TB[:], t0, t1), nt), cbc(0, nt))
                            nc.vector.tensor_add(
                                sl(res[:], t0, t1), sl(res[:], t0, t1),
                                tmpb[:, :nt * H])
                        # store T_k chunk as bf16 (cast DMA) for AllGather
                        if hop <= 4:
                            nc.gpsimd.dma_start(
                                shard_b[hop - 1][t0 * 128:t1 * 128, :]
                                .rearrange("(t p) j -> p t j", p=128),
                                sl(cur[:], t0, t1)
                                .rearrange("p (t j) -> p t j", j=H))
                    if hop <= 4:
                        nc.gpsimd.collective_compute(
                            "AllGather", mybir.AluOpType.bypass,
                            replica_groups=[list(range(NCORES))],
                            ins=[shard_b[hop - 1][:].opt()],
                            outs=[full_b[hop - 1][:].opt()],
                        )

                # epilogue: res complete; TA/TB are dead scratch now
                def res3(ap):
                    return ap.rearrange("p (t j) -> p t j", j=H)

                gall = res[:].rearrange("p (t g c) -> p t g c", g=GROUPS,
                                        c=GC)

                def pbc(c0):
                    return (par_sb[:, c0:c0 + 4]
                            .rearrange("p g -> p () g ()")
                            .to_broadcast([128, TILES, GROUPS, GC]))

                nc.vector.tensor_mul(gall, gall, pbc(24))
                nc.vector.tensor_add(gall, gall, pbc(28))
                nc.vector.tensor_mul(TB[:], res[:], res[:])
                ssq = small.tile([128, TILES], f32, tag="ssq")
                nc.vector.tensor_reduce(
                    ssq[:], res3(TB[:]), mybir.AxisListType.X,
                    mybir.AluOpType.add)
                rms = small.tile([128, TILES], f32, tag="rms")
                sq = small.tile([128, TILES], f32, tag="sqr")
                nc.scalar.activation(
                    sq[:], ssq[:], mybir.ActivationFunctionType.Sqrt,
                    bias=par_sb[:, 32:33], scale=1.0 / H)
                nc.vector.reciprocal(rms[:], sq[:])
                nc.vector.tensor_mul(
                    res3(TA[:]), res3(res[:]),
                    rms[:].rearrange("p (t o) -> p t o", o=1)
                    .to_broadcast([128, TILES, H]))
                nc.vector.tensor_mul(
                    res3(TA[:]), res3(TA[:]),
                    rw_sb[:].rearrange("p (o j) -> p o j", o=1)
                    .to_broadcast([128, TILES, H]))
                # SiLU = x * sigmoid(x)
                nc.scalar.activation(
                    TB[:], TA[:], mybir.ActivationFunctionType.Sigmoid)
                nc.vector.tensor_mul(TA[:], TA[:], TB[:])
                nc.sync.dma_start(
                    out_ext[:].rearrange("(t p) j -> p t j", p=128),
                    res3(TA[:]))

    nc.compile()
    _cache[key] = nc
    return nc


def _order_profiles(profs, counts):
    """Greedy nearest-neighbor chain over distinct profile rows (L1)."""
    np_, _ = profs.shape
    visited = np.zeros(np_, bool)
    cur = int(np.argmax(counts))
    order = [cur]
    visited[cur] = True
    for _ in range(np_ - 1):
        d = np.abs(profs - profs[cur]).sum(axis=1).astype(np.float64)
        d[visited] = np.inf
        cur = int(np.argmin(d))
        order.append(cur)
        visited[cur] = True
    return np.array(order)


def _prep(x, edge_weight_norm, edge_index):
    src = np.asarray(edge_index[0]).astype(np.int64)
    dst = np.asarray(edge_index[1]).astype(np.int64)
    ew = np.asarray(edge_weight_norm, dtype=np.float32)
    E = src.shape[0]

    # pass 1: degree-sorted round-robin deal fixes each node's core (and
    # hence its pair-window = core//4 as a gather source)
    deg = np.bincount(dst, minlength=N)
    order1 = np.argsort(-deg, kind='stable')
    node_core = np.empty(N, np.int64)
    node_core[order1] = np.arange(N) % NCORES

    # profiles: in-edge counts split by src window (fixed by pass 1)
    src_win = node_core[src] // 4
    prof = np.zeros((N, NWIN), np.int32)
    np.add.at(prof, (dst, src_win), 1)

    # pass 2: within each core, order nodes by a shared profile-bucket
    # chain so tiles hold near-identical (c0, c1) profiles, aligned
    # across cores by quantile
    uniq, uinv, ucnt = np.unique(prof, axis=0, return_inverse=True,
                                 return_counts=True)
    porder = _order_profiles(uniq.astype(np.int64), ucnt)
    prank = np.empty(len(uniq), np.int64)
    prank[porder] = np.arange(len(uniq))
    node_rank = prank[uinv]

    perm_pos = np.empty(N, np.int64)
    for c in range(NCORES):
        nodes = np.flatnonzero(node_core == c)
        o = np.argsort(node_rank[nodes], kind='stable')
        perm_pos[nodes[o]] = c * SHARD + np.arange(len(nodes))

    dst_p = perm_pos[dst]
    dst_core = dst_p // SHARD
    dst_local = dst_p - dst_core * SHARD
    tile_id = dst_local // 128
    part_id = dst_local % 128
    src_p = perm_pos[src]
    prow = src_p // 2
    win_id = prow // PWIN
    win_rel = (prow - win_id * PWIN).astype(np.int16)
    sub = src_p % 2

    cnt = np.zeros((NCORES, TILES, 128, NWIN), np.int32)
    np.add.at(cnt, (dst_core, tile_id, part_id, win_id), 1)
    R_tw = cnt.max(axis=(0, 2)).astype(np.int64)  # [TILES, NWIN]

    # greedy chunks of consecutive tiles, uniform per-window rounds
    chunks = []
    q_list = []
    cur = [0]
    q = R_tw[0].copy()
    for t in range(1, TILES):
        q2 = np.maximum(q, R_tw[t])
        nt = len(cur)
        waste = (nt + 1) * q2.sum() - (nt * q.sum() + R_tw[t].sum())
        if nt + 1 <= NTMAX and (nt + 1) * q2.sum() <= CAPR and waste <= 3:
            cur.append(t)
            q = q2
        else:
            chunks.append(cur)
            q_list.append(q)
            cur = [t]
            q = R_tw[t].copy()
    chunks.append(cur)
    q_list.append(q)
    Q = np.stack(q_list)  # [nchunk, NWIN]
    nch = len(chunks)
    col0w = np.zeros((nch, NWIN), np.int64)
    run = 0
    for s in range(nch):
        for w in range(NWIN):
            col0w[s, w] = run
            run += len(chunks[s]) * int(Q[s, w])
    TR = int(run)

    chunk_of = np.zeros(TILES, np.int64)
    tl_of = np.zeros(TILES, np.int64)
    for s, ts in enumerate(chunks):
        for i, t in enumerate(ts):
            chunk_of[t] = s
            tl_of[t] = i

    # rank of edge within its (core, tile, part, window) group
    key = (((dst_core * TILES + tile_id) * 128 + part_id) * NWIN + win_id)
    o = np.argsort(key, kind='stable')
    ks = key[o]
    starts = np.r_[0, np.flatnonzero(np.diff(ks)) + 1]
    group_len = np.diff(np.r_[starts, E])
    rank_sorted = np.arange(E) - np.repeat(starts, group_len)
    erank = np.empty(E, np.int64)
    erank[o] = rank_sorted

    s_of = chunk_of[tile_id]
    gcol = (col0w[s_of, win_id] + tl_of[tile_id] * Q[s_of, win_id] + erank)

    ew_all = []
    idxw_all = []
    for c in range(NCORES):
        m = dst_core == c
        ewf = np.zeros((128, 2 * TR), np.float32)
        ewf[part_id[m], 2 * gcol[m] + sub[m]] = ew[m]
        idx_flat = np.zeros(TR * 128, np.int16)
        idx_flat[gcol[m] * 128 + part_id[m]] = win_rel[m]
        iw = np.zeros((128, 8 * TR), np.int16)
        for s in range(nch):
            nt = len(chunks[s])
            for w in range(NWIN):
                rw_ = nt * int(Q[s, w])
                if rw_ == 0:
                    continue
                cb = int(col0w[s, w])
                seg = idx_flat[cb * 128:(cb + rw_) * 128]
                iw[:, 8 * cb:8 * (cb + rw_)] = np.tile(
                    seg.reshape(-1, 16).T, (8, 1))
        ew_all.append(ewf)
        idxw_all.append(iw)

    x_full = np.zeros((FULL, H), np.float32)
    x_full[perm_pos] = np.asarray(x, np.float32)
    x_own_all = [x_full[c * SHARD:(c + 1) * SHARD] for c in range(NCORES)]
    inv = np.full(FULL, -1, np.int64)
    inv[perm_pos] = np.arange(N)
    schedule = (chunks, Q, col0w, TR)
    return x_full, x_own_all, idxw_all, ew_all, schedule, inv


def _np_bf16(a):
    import ml_dtypes
    return np.asarray(a, np.float32).astype(ml_dtypes.bfloat16)


def kernel(x, edge_weight_norm, cheb_coeffs, group_scale, group_bias,
           rms_weight, edge_index):
    from concourse.bass_utils import run_bass_kernel_spmd

    x = np.asarray(x, np.float32)
    assert x.shape == (N, H)
    x_full, x_own_all, idxw_all, ew_all, schedule, inv = _prep(
        x, edge_weight_norm, edge_index)

    params = np.zeros((128, 40), np.float32)
    params[:, 32] = RMS_EPS
    cheb = np.asarray(cheb_coeffs, np.float32)      # [4, K+1]
    params[:, :24] = cheb.T.reshape(1, 24)          # k-major: col = 4k+g
    params[:, 24:28] = np.asarray(group_scale, np.float32).reshape(1, 4)
    params[:, 28:32] = np.asarray(group_bias, np.float32).reshape(1, 4)
    rmsw = np.tile(np.asarray(rms_weight, np.float32).reshape(1, H), (128, 1))

    repeat = int(os.environ.get("KERNEL_REPEAT", "1"))
    nc = _build(schedule, repeat=repeat)

    x_fullb = _np_bf16(x_full)
    in_maps = []
    for c in range(NCORES):
        in_maps.append({
            "x_fullb": x_fullb,
            "x_own": x_own_all[c],
            "idx_in": idxw_all[c],
            "ew_in": _np_bf16(ew_all[c]),
            "par_in": params,
            "rw_in": rmsw,
        })
    res = run_bass_kernel_spmd(nc, in_maps, list(range(NCORES)))
    out_shards = np.stack([res.results[c]["out"] for c in range(NCORES)],
                          axis=0)
    out_full = out_shards.reshape(FULL, H)
    out = np.empty((N, H), np.float32)
    mask = inv >= 0
    out[inv[mask]] = out_full[mask]
    return out
